# revision 1
# baseline (speedup 1.0000x reference)
"""AFT (attention-free transformer) block on 8 TRN2 NeuronCores.

Reference computation (T=2048, B=4, D=1024):
    qkv = data @ W_qkv + b_qkv ; q,k,v = split(qkv)
    num = exp(pb - max_pb) @ (exp(k - max_k) * v)    (contraction over key pos j)
    den = exp(pb - max_pb) @ exp(k - max_k)
    out = (sigmoid(q) * num / den) @ W_out + b_out
The max shifts cancel exactly in num/den and the value ranges are tiny
(|k| <~ 4, |pb| <~ 0.12), so the kernel drops them. Compute is bf16 with
fp32 PSUM accumulation (rel err ~4e-3 vs the fp32 reference).

Sharding: hybrid (sequence-half x batch). Core r = 2b + h owns batch b and
query rows i in [h*1024, (h+1)*1024). Each core projects q/k/v for its own
1024 tokens; the 8 cores all-gather exp(k) and exp(k)*v (bf16, two pipelined
j-half chunks); each core then reads back ONLY its batch's slice of the
gathered buffer (8MB instead of 32MB) via indirect DMAs whose row indices
are a per-core host input (gidx) — the SPMD graph stays uniform while the
rank blocks read differ per core. sigmoid(q) needs no data movement at all
because batch is fixed per core.

Pipeline (driven by the ~160us AllGather wire time, the binding constraint):
  - k/v columns are projected first (q deferred) so AG chunk 0 triggers
    ~55us in; the two 2MB collectives then stream back to back.
  - num/den accumulation is split into three passes so the PE never waits
    on the wire: B1 consumes the core's OWN eight j-tiles straight out of
    its local cc_in staging buffers (no collective dependency), B2 adds the
    four gathered chunk-0 other-half tiles once AG0 lands, B3 adds the four
    chunk-1 other-half tiles once AG1 lands. Partials are spilled to SBUF
    as bf16 between passes and merged back into PSUM with identity-matmuls
    (PSUM += I.T @ spill), keeping the DVE epilogue chain short; groups run
    in pairs so one group's matmuls hide the other's epilogue. The host
    permutes each core's pbT slice into slot order so the j-accumulation
    order matches the tile sources uniformly across cores.
  - every matmul reuses one stationary (lhsT) load for 2-4 N=512 moving
    passes (ldw-opt is off in this compile config, so LDWEIGHTS serialize).
  - y is token-major; the [d, i] transposes for the output projection run
    as PE transposes through 2 dedicated PSUM banks.
"""

import numpy as np
import ml_dtypes

from concourse import bacc, bass, mybir, tile
from concourse.bass_utils import run_bass_kernel_spmd
from concourse.masks import make_identity

BF16 = mybir.dt.bfloat16
F32 = mybir.dt.float32
I32 = mybir.dt.int32
AF = mybir.ActivationFunctionType

N_CORES = 8
T, B, D = 2048, 4, 1024
TOK = 1024                 # tokens per core: 1024 query rows of one batch
KT = D // 128              # 8 contraction tiles for d
NG = TOK // 128            # 8 query-tile groups

_cache = {}


def build(with_qkv_bias: bool, with_out_bias: bool):
    nc = bacc.Bacc(None, target_bir_lowering=False)

    dataT_d = nc.dram_tensor("dataT", [D, TOK], BF16, kind="ExternalInput")
    wkv_d = nc.dram_tensor("wkv", [D, 2 * D], BF16, kind="ExternalInput")
    wq_d = nc.dram_tensor("wq", [D, D], BF16, kind="ExternalInput")
    pbT_d = nc.dram_tensor("pbT", [T, TOK], BF16, kind="ExternalInput")
    wout_d = nc.dram_tensor("wout", [D, D], BF16, kind="ExternalInput")
    gidx_d = nc.dram_tensor("gidx", [128, 24], I32, kind="ExternalInput")
    out_d = nc.dram_tensor("out", [TOK, D], F32, kind="ExternalOutput")
    if with_qkv_bias:
        bkv_d = nc.dram_tensor("bkv", [1, 2 * D], BF16, kind="ExternalInput")
        bq_d = nc.dram_tensor("bq", [1, D], BF16, kind="ExternalInput")
    if with_out_bias:
        bout_d = nc.dram_tensor("bout", [1, D], BF16, kind="ExternalInput")

    with tile.TileContext(nc) as tc:
        with (
            tc.tile_pool(name="persist", bufs=1) as pp,
            tc.tile_pool(name="psum", bufs=6, space="PSUM") as psp,
            tc.tile_pool(name="psum_tr", bufs=2, space="PSUM") as pstr,
            tc.tile_pool(name="dram", bufs=1, space="DRAM") as dram,
        ):
            # ---- persistent SBUF tensors ----
            ident = pp.tile([128, 128], BF16, name="ident", tag="ident")
            make_identity(nc, ident[:])
            gidx = pp.tile([128, 24], I32, name="gidx", tag="gidx")
            wout = [pp.tile([128, D], BF16, name=f"wout{k}", tag=f"wout{k}")
                    for k in range(KT)]
            pbe = [pp.tile([128, TOK], BF16, name=f"pbe{t}", tag=f"pbe{t}")
                   for t in range(T // 128)]
            sq_t = [pp.tile([128, D], BF16, name=f"sq{m}", tag=f"sq{m}")
                    for m in range(NG)]
            if with_qkv_bias or with_out_bias:
                ones1 = pp.tile([1, 128], BF16, name="ones1", tag="ones1")
                nc.gpsimd.memset(ones1[:], 1.0)
            if with_qkv_bias:
                bkv = pp.tile([1, 2 * D], BF16, name="bkv", tag="bkv")
                nc.sync.dma_start(bkv[:], bkv_d[:])
                bq = pp.tile([1, D], BF16, name="bq", tag="bq")
                nc.sync.dma_start(bq[:], bq_d[:])
            if with_out_bias:
                bout = pp.tile([1, D], BF16, name="bout", tag="bout")
                nc.sync.dma_start(bout[:], bout_d[:])

            # collective bounce buffers: two token-half chunks of [ek | ekv]
            cc_in = [dram.tile([TOK, D], BF16, name=f"cc_in{x}") for x in range(2)]
            cc_out = [dram.tile([N_CORES * TOK, D], BF16, name=f"cc_out{x}",
                                addr_space="Shared") for x in range(2)]

            # ---- phase A: qkv projection, k/v first ----
            with tc.tile_pool(name="phaseA", bufs=1) as pa:
                dataT = [pa.tile([128, TOK], BF16, name=f"dataT{k}",
                                 tag=f"dataT{k}") for k in range(KT)]
                wkv = [pa.tile([128, 2 * D], BF16, name=f"wkv{k}",
                               tag=f"wkv{k}") for k in range(KT)]
                wq = [pa.tile([128, D], BF16, name=f"wq{k}", tag=f"wq{k}")
                      for k in range(KT)]
                # kv weights first so pass 1 can start after ~6MB of DMA
                for k in range(KT):
                    nc.sync.dma_start(dataT[k][:], dataT_d[k * 128:(k + 1) * 128, :])
                    nc.sync.dma_start(wkv[k][:], wkv_d[k * 128:(k + 1) * 128, :])
                for k in range(KT):
                    nc.sync.dma_start(wq[k][:], wq_d[k * 128:(k + 1) * 128, :])

                # pass 1: k and v chunks -> exp(k), exp(k)*v -> cc_in -> AG
                for m in range(NG):  # token tile
                    ek = pa.tile([128, D], BF16, name=f"ek{m}", tag="ek", bufs=3)
                    vv = pa.tile([128, D], BF16, name=f"vv{m}", tag="vv", bufs=3)
                    ekv = pa.tile([128, D], BF16, name=f"ekv{m}", tag="ekv", bufs=3)
                    ps = [psp.tile([128, 512], F32, name=f"ps{m}_{i}",
                                   tag="ps") for i in range(4)]
                    for k in range(KT):
                        for i in range(4):
                            nc.tensor.matmul(
                                ps[i][:], dataT[k][:, m * 128:(m + 1) * 128],
                                wkv[k][:, i * 512:(i + 1) * 512],
                                start=(k == 0),
                                stop=(k == KT - 1 and not with_qkv_bias),
                            )
                    if with_qkv_bias:
                        for i in range(4):
                            nc.tensor.matmul(
                                ps[i][:], ones1[:], bkv[:, i * 512:(i + 1) * 512],
                                start=False, stop=True,
                            )
                    for i in range(2):
                        nc.scalar.activation(
                            ek[:, i * 512:(i + 1) * 512], ps[i][:], AF.Exp)
                        nc.vector.tensor_copy(
                            vv[:, i * 512:(i + 1) * 512], ps[2 + i][:])
                    nc.vector.tensor_mul(ekv[:], ek[:], vv[:])
                    # chunk x = m//4 holds token rows [x*512,(x+1)*512):
                    # layout [ek half | ekv half]
                    x, mm = m // 4, m % 4
                    nc.sync.dma_start(
                        cc_in[x][mm * 128:(mm + 1) * 128, :], ek[:])
                    nc.sync.dma_start(
                        cc_in[x][512 + mm * 128:512 + (mm + 1) * 128, :], ekv[:])
                    if m in (3, 7):
                        nc.gpsimd.collective_compute(
                            "AllGather", mybir.AluOpType.bypass,
                            replica_groups=[list(range(N_CORES))],
                            ins=[cc_in[m // 4][:].opt()],
                            outs=[cc_out[m // 4][:].opt()],
                        )

                # exp(pbT) — loads ride behind the cc_in stores, done ~mid-AG
                for t in range(T // 128):
                    praw = pa.tile([128, TOK], BF16, name=f"praw{t}", tag="praw",
                                   bufs=4)
                    nc.sync.dma_start(praw[:], pbT_d[t * 128:(t + 1) * 128, :])
                    nc.scalar.activation(pbe[t][:], praw[:], AF.Exp)

                # pass 2: q chunks + sigmoid — overlaps the collectives;
                # the second half is emitted after the even pass to fill the
                # wait-for-AG1 bubble
                def q_group(m):
                    sq = sq_t[m]
                    ps = [psp.tile([128, 512], F32, name=f"psq{m}_{i}",
                                   tag="ps") for i in range(2)]
                    for k in range(KT):
                        for i in range(2):
                            nc.tensor.matmul(
                                ps[i][:], dataT[k][:, m * 128:(m + 1) * 128],
                                wq[k][:, i * 512:(i + 1) * 512],
                                start=(k == 0),
                                stop=(k == KT - 1 and not with_qkv_bias),
                            )
                    if with_qkv_bias:
                        for i in range(2):
                            nc.tensor.matmul(
                                ps[i][:], ones1[:], bq[:, i * 512:(i + 1) * 512],
                                start=False, stop=True,
                            )
                    for i in range(2):
                        nc.scalar.activation(
                            sq[:, i * 512:(i + 1) * 512], ps[i][:], AF.Sigmoid)

                for m in range(NG):
                    q_group(m)

                # wout/gidx: needed only by phase B, much later
                nc.sync.dma_start(gidx[:], gidx_d[:])
                for k in range(KT):
                    nc.sync.dma_start(wout[k][:], wout_d[k * 128:(k + 1) * 128, :])

            # ---- phase B: num/den + y + output projection ----
            # pbe tiles are SLOT-ordered (host-permuted pbT rows):
            #   s in [0,8):   chunk-0 j-tile s (absolute half s//4, tile s%4)
            #   s in [8,12):  chunk-1 OWN-half tile s-8   (local in cc_in[1])
            #   s in [12,16): chunk-1 OTHER-half tile s-12 (needs AG chunk 1)
            # gidx cols: 0-15 chunk-0 (half*8+u), 16-23 chunk-1-other
            # (half*4+o). The own-half chunk-1 tiles are read straight from
            # this core's cc_in[1], before the second collective lands.
            with tc.tile_pool(name="phaseB", bufs=1) as pbp:
                def gload(x, cols, nt, tagp):
                    tiles = []
                    for u in range(nt):
                        g = pbp.tile([128, 2048], BF16, name=f"ekg{tagp}{u}",
                                     tag="ekg", bufs=12)
                        for half in range(2):
                            col = cols + half * nt + u
                            nc.gpsimd.indirect_dma_start(
                                out=g[:, half * D:(half + 1) * D],
                                out_offset=None,
                                in_=cc_out[x][:],
                                in_offset=bass.IndirectOffsetOnAxis(
                                    ap=gidx[:, col:col + 1], axis=0),
                            )
                        tiles.append(g)
                    return tiles

                spill = {}

                def nd_mms(m2, tiles, merge_sp, tagp):
                    pn = [psp.tile([128, 512], F32, name=f"pn{tagp}{m2}{i}",
                                   tag="ps") for i in range(2)]
                    pd = [psp.tile([128, 512], F32, name=f"pd{tagp}{m2}{i}",
                                   tag="ps") for i in range(2)]
                    nt = len(tiles)
                    for u in range(nt):
                        tile_u, slot = tiles[u]
                        pb_t = pbe[slot]
                        for i in range(2):
                            nc.tensor.matmul(
                                pn[i][:], pb_t[:, m2 * 128:(m2 + 1) * 128],
                                tile_u[:, D + i * 512:D + (i + 1) * 512],
                                start=(u == 0),
                                stop=(u == nt - 1 and merge_sp is None))
                            nc.tensor.matmul(
                                pd[i][:], pb_t[:, m2 * 128:(m2 + 1) * 128],
                                tile_u[:, i * 512:(i + 1) * 512],
                                start=(u == 0),
                                stop=(u == nt - 1 and merge_sp is None))
                    if merge_sp is not None:
                        for i in range(2):
                            nc.tensor.matmul(
                                pn[i][:], ident[:],
                                merge_sp[:, i * 512:(i + 1) * 512],
                                start=False, stop=True)
                            nc.tensor.matmul(
                                pd[i][:], ident[:],
                                merge_sp[:, D + i * 512:D + (i + 1) * 512],
                                start=False, stop=True)
                    return pn, pd

                def to_spill(m2, pn, pd):
                    if m2 in spill:
                        sp = spill[m2]
                    else:
                        sp = pbp.tile([128, 4 * 512], BF16, name=f"sp{m2}",
                                      tag=f"sp{m2}")
                        spill[m2] = sp
                    for i in range(2):
                        nc.scalar.copy(sp[:, i * 512:(i + 1) * 512], pn[i][:])
                        nc.scalar.copy(
                            sp[:, D + i * 512:D + (i + 1) * 512], pd[i][:])

                # pass B1: BOTH chunks' own-half tiles, straight from this
                # core's cc_in — runs while the collectives are on the wire
                own = []
                for x in range(2):
                    for o in range(4):
                        g = pbp.tile([128, 2048], BF16, name=f"ekgo{x}{o}",
                                     tag="ekg", bufs=12)
                        nc.sync.dma_start(
                            g[:, :D], cc_in[x][o * 128:(o + 1) * 128, :])
                        nc.sync.dma_start(
                            g[:, D:],
                            cc_in[x][512 + o * 128:512 + (o + 1) * 128, :])
                        own.append((g, 8 * x + o))
                for pair in range(NG // 2):
                    g0, g1 = 2 * pair, 2 * pair + 1
                    a0 = nd_mms(g0, own, None, "A")
                    to_spill(g0, *a0)
                    a1 = nd_mms(g1, own, None, "A")
                    to_spill(g1, *a1)

                # pass B2: chunk-0 other-half tiles (after AG chunk 0),
                # merging the B1 partials back in on the PE
                ekg0 = [(g, 4 + o) for o, g in enumerate(gload(0, 0, 4, "z"))]
                for pair in range(NG // 2):
                    g0, g1 = 2 * pair, 2 * pair + 1
                    a0 = nd_mms(g0, ekg0, spill[g0], "B")
                    to_spill(g0, *a0)
                    a1 = nd_mms(g1, ekg0, spill[g1], "B")
                    to_spill(g1, *a1)

                # pass B3: chunk-1 other-half tiles (after AG chunk 1),
                # epilogue, transpose, output projection
                ekg1 = [(g, 12 + o) for o, g in enumerate(gload(1, 8, 4, "o"))]

                def tail(m2, yv):
                    yT = [pbp.tile([128, 128], BF16, name=f"yT{m2}_{k}",
                                   tag=f"yT{k}", bufs=2) for k in range(KT)]
                    for k in range(KT):
                        pt = pstr.tile([128, 128], BF16, name=f"pt{m2}{k}",
                                       tag="tr")
                        nc.tensor.transpose(
                            pt[:], yv[:, k * 128:(k + 1) * 128], ident[:])
                        nc.vector.tensor_copy(yT[k][:], pt[:])
                    po = [psp.tile([128, 512], F32, name=f"po{m2}_{n}", tag="ps")
                          for n in range(2)]
                    for k in range(KT):
                        for n in range(2):
                            nc.tensor.matmul(
                                po[n][:], yT[k][:],
                                wout[k][:, n * 512:(n + 1) * 512],
                                start=(k == 0),
                                stop=(k == KT - 1 and not with_out_bias))
                    if with_out_bias:
                        for n in range(2):
                            nc.tensor.matmul(
                                po[n][:], ones1[:], bout[:, n * 512:(n + 1) * 512],
                                start=False, stop=True)
                    for n in range(2):
                        osb = pbp.tile([128, 512], F32, name=f"osb{m2}_{n}",
                                       tag="osb", bufs=4)
                        nc.scalar.copy(osb[:], po[n][:])
                        nc.sync.dma_start(
                            out_d[m2 * 128:(m2 + 1) * 128,
                                  n * 512:(n + 1) * 512], osb[:])

                def odd_group(m2):
                    pn, pd = nd_mms(m2, ekg1, spill[m2], "C")
                    y = pbp.tile([128, D], BF16, name=f"y{m2}", tag="y", bufs=3)
                    for i in range(2):
                        tn = pbp.tile([128, 512], F32, name=f"tn{m2}{i}",
                                      tag="tn", bufs=3)
                        rec = pbp.tile([128, 512], F32, name=f"rc{m2}{i}",
                                       tag="rc", bufs=3)
                        nc.vector.reciprocal_approx_fast(rec[:], pd[i][:])
                        nc.vector.tensor_mul(tn[:], pn[i][:], rec[:])
                        nc.vector.tensor_mul(
                            y[:, i * 512:(i + 1) * 512], tn[:],
                            sq_t[m2][:, i * 512:(i + 1) * 512])
                    return y

                for pair in range(NG // 2):
                    g0, g1 = 2 * pair, 2 * pair + 1
                    y0 = odd_group(g0)
                    y1 = odd_group(g1)
                    tail(g0, y0)
                    tail(g1, y1)

    nc.compile()
    return nc


def _prep_inputs(data, W_qkv, b_qkv, pos_bias_param, W_out, b_out):
    bf = ml_dtypes.bfloat16
    data = np.asarray(data, np.float32)
    W_qkv = np.asarray(W_qkv, np.float32)
    b_qkv = np.asarray(b_qkv, np.float32)
    pos_bias_param = np.asarray(pos_bias_param, np.float32)
    W_out = np.asarray(W_out, np.float32)
    b_out = np.asarray(b_out, np.float32)

    with_qkv_bias = bool(np.any(b_qkv))
    with_out_bias = bool(np.any(b_out))

    wq = np.ascontiguousarray(W_qkv[:, :D]).astype(bf)
    wkv = np.ascontiguousarray(W_qkv[:, D:]).astype(bf)
    wout = W_out.astype(bf)
    pbT = np.ascontiguousarray(pos_bias_param.T)  # [j, i]

    p = np.arange(128)
    in_maps = []
    for r in range(N_CORES):
        b, h = r // 2, r % 2
        isl = slice(h * TOK, (h + 1) * TOK)
        dT = np.ascontiguousarray(
            data[isl, b, :].T).astype(bf)                    # [d_in, tok]
        pbT_c = np.ascontiguousarray(pbT[:, isl]).astype(bf)  # [j, i_loc]
        # slot-permute pbT rows: per chunk x, slots 0-3 = own half,
        # 4-7 = other half; pbe slot s = 8*x + within-chunk slot
        rows = []
        for s in range(16):
            x, ss = s // 8, s % 8
            hp = h if ss < 4 else 1 - h
            j0 = hp * 1024 + 512 * x + (ss % 4) * 128
            rows.append(pbT_c[j0:j0 + 128])
        pbT_c = np.ascontiguousarray(np.concatenate(rows, axis=0))
        # gidx: cols x*8 + half*4 + o -> chunk-x other-half tile o
        gidx = np.zeros((128, 24), np.int32)
        for x in range(2):
            for half in range(2):
                for o in range(4):
                    base = (2 * b + (1 - h)) * 1024 + half * 512 + o * 128
                    gidx[:, x * 8 + half * 4 + o] = base + p
        m = {"dataT": dT, "wq": wq, "wkv": wkv, "pbT": pbT_c, "wout": wout,
             "gidx": gidx}
        if with_qkv_bias:
            m["bq"] = np.ascontiguousarray(b_qkv[:D]).reshape(1, D).astype(bf)
            m["bkv"] = np.ascontiguousarray(b_qkv[D:]).reshape(1, 2 * D).astype(bf)
        if with_out_bias:
            m["bout"] = b_out.reshape(1, D).astype(bf)
        in_maps.append(m)
    return in_maps, with_qkv_bias, with_out_bias


def run(data, W_qkv, b_qkv, pos_bias_param, W_out, b_out, **spmd_kwargs):
    in_maps, wb, ob = _prep_inputs(data, W_qkv, b_qkv, pos_bias_param, W_out, b_out)
    key = (wb, ob)
    if key not in _cache:
        _cache[key] = build(wb, ob)
    nc = _cache[key]
    res = run_bass_kernel_spmd(nc, in_maps, core_ids=list(range(N_CORES)),
                               **spmd_kwargs)
    out = np.empty((T, B, D), np.float32)
    for r in range(N_CORES):
        b, h = r // 2, r % 2
        out[h * TOK:(h + 1) * TOK, b, :] = res.results[r]["out"]
    return out, res


def kernel(data, W_qkv, b_qkv, pos_bias_param, W_out, b_out):
    out, _ = run(data, W_qkv, b_qkv, pos_bias_param, W_out, b_out)
    return out



# revision 4
# speedup vs baseline: 1.4543x; 1.4543x over previous
"""AFT (attention-free transformer) block on 8 TRN2 NeuronCores.

Reference computation (T=2048, B=4, D=1024):
    qkv = data @ W_qkv + b_qkv ; q,k,v = split(qkv)
    num = exp(pb - max_pb) @ (exp(k - max_k) * v)    (contraction over key pos j)
    den = exp(pb - max_pb) @ exp(k - max_k)
    out = (sigmoid(q) * num / den) @ W_out + b_out
The max shifts cancel exactly in num/den so the kernel drops them.

Sharding: hybrid (sequence-half x batch). Core r = 2b + h owns batch b and
query rows i in [h*1024, (h+1)*1024). Each core projects q/k/v for its own
1024 tokens; k-half exchange is a PAIRWISE AllGather (replica groups
[[0,1],[2,3],[4,5],[6,7]]) of fp8 exp(k) / exp(k)*v, two pipelined chunks.

Precision trick: exp(pb) = 1 + r with r = expm1(pb) in [-0.09, 0.1], so
    num = Snum + r @ ekv,   Snum[d] = sum_j ekv[j,d]   (i-independent)
    den = Sden + r @ ek
The S sums are computed in bf16/fp32 (ones-matmul on the PE, then a
K=1-matmul transpose into per-partition columns); the big TxT einsum runs
on the small correction term with BOTH operands fp8e4 and
perf_mode=DoubleRow (K virtualized to 256, ~2x PE throughput). The fp8
quantization error only touches the ~2% correction, keeping overall rel
err ~5e-3. r is pre-scaled by 64 on the host (epilogue rescales by 1/64).

Everything downstream of the pb einsum is TRANSPOSED ([d,i] layout): the q
projection emits sigmoid(q)^T directly (lhsT = W_q), num/den come out of
the DoubleRow matmuls as [d_chunk, i], and the output projection consumes
y^T as lhsT directly -- no PE transposes, no spill/merge passes.

Timeline per core: kv projection (8 token tiles, chunk AGs fired at tiles
3/7) -> S finalize -> q^T projection + sigmoid (covers AG wire time) ->
num/den DoubleRow accumulation (16 j-tiles as 8 pairs, single PSUM pass)
-> epilogue (reciprocal, sigmoid multiply) -> output projection.
"""

import numpy as np
import ml_dtypes

from concourse import bacc, bass, mybir, tile
from concourse.bass_utils import run_bass_kernel_spmd

BF16 = mybir.dt.bfloat16
F32 = mybir.dt.float32
F8 = mybir.dt.float8e4
AF = mybir.ActivationFunctionType
ALU = mybir.AluOpType
DR = mybir.MatmulPerfMode.DoubleRow

N_CORES = 8
T, B, D = 2048, 4, 1024
TOK = 1024                 # tokens per core: 1024 query rows of one batch
KT = D // 128              # 8 contraction tiles for d_in
NG = TOK // 128            # 8 token/query tile groups
PAIRS = T // 256           # 8 j-block pairs (DoubleRow processes 256 j rows)
SCALE = 64.0               # host pre-scale on expm1(pb) for fp8 range
PAIR_GROUPS = [[0, 1], [2, 3], [4, 5], [6, 7]]

_cache = {}


def build(with_qkv_bias: bool, with_out_bias: bool):
    nc = bacc.Bacc(None, target_bir_lowering=False)

    dataT_d = nc.dram_tensor("dataT", [D, TOK], BF16, kind="ExternalInput")
    wkv_d = nc.dram_tensor("wkv", [D, 2 * D], BF16, kind="ExternalInput")
    wq_d = nc.dram_tensor("wq", [D, D], BF16, kind="ExternalInput")
    pbr_d = nc.dram_tensor("pbr", [TOK, 2048], F8, kind="ExternalInput")
    wout_d = nc.dram_tensor("wout", [D, D], BF16, kind="ExternalInput")
    out_d = nc.dram_tensor("out", [TOK, D], F32, kind="ExternalOutput")
    if with_qkv_bias:
        bkv_d = nc.dram_tensor("bkv", [1, 2 * D], BF16, kind="ExternalInput")
        bqt_d = nc.dram_tensor("bqt", [128, KT], F32, kind="ExternalInput")
    if with_out_bias:
        bout_d = nc.dram_tensor("bout", [1, D], BF16, kind="ExternalInput")

    with tile.TileContext(nc) as tc:
        with (
            tc.tile_pool(name="persist", bufs=1) as pp,
            tc.tile_pool(name="psum", bufs=6, space="PSUM") as psp,
            tc.tile_pool(name="psum_s", bufs=1, space="PSUM") as pss,
            tc.tile_pool(name="dram", bufs=1, space="DRAM") as dram,
        ):
            # ---- persistent SBUF tensors ----
            onescol = pp.tile([128, 1], BF16, name="onescol", tag="onescol")
            nc.gpsimd.memset(onescol[:], 1.0)
            ones11 = pp.tile([1, 1], F32, name="ones11", tag="ones11")
            nc.gpsimd.memset(ones11[:], 1.0)
            # ST cols 0-7: Sden per d-chunk; cols 8-15: Snum
            ST = pp.tile([128, 16], F32, name="ST", tag="ST")
            srow_d = pp.tile([1, D], F32, name="srow_d", tag="srow_d")
            srow_n = pp.tile([1, D], F32, name="srow_n", tag="srow_n")
            wout = [pp.tile([128, D], BF16, name=f"wout{k}", tag=f"wout{k}")
                    for k in range(KT)]
            pbr = [pp.tile([128, 2, TOK], F8, name=f"pbr{u}", tag=f"pbr{u}")
                   for u in range(PAIRS)]
            sq_t = [pp.tile([128, TOK], BF16, name=f"sq{c}", tag=f"sq{c}")
                    for c in range(KT)]
            if with_qkv_bias or with_out_bias:
                ones1r = pp.tile([1, 128], BF16, name="ones1r", tag="ones1r")
                nc.gpsimd.memset(ones1r[:], 1.0)
            if with_qkv_bias:
                bkv = pp.tile([1, 2 * D], BF16, name="bkv", tag="bkv")
                nc.sync.dma_start(bkv[:], bkv_d[:])
                bqt = pp.tile([128, KT], F32, name="bqt", tag="bqt")
                nc.sync.dma_start(bqt[:], bqt_d[:])
            if with_out_bias:
                bout = pp.tile([1, D], BF16, name="bout", tag="bout")
                nc.sync.dma_start(bout[:], bout_d[:])

            # S accumulators: row 0 = Sden, row 32 = Snum (PE col-strips)
            s2 = [pss.tile([64, 512], F32, name=f"s2_{ih}", tag=f"s2_{ih}")
                  for ih in range(2)]

            # collective bounce buffers, fp8, one chunk per 512 own tokens:
            # rows 0:256 = ek pairs (q=0,1), 256:512 = ekv pairs; within a
            # pair row-block the two 128-j subtiles sit in column halves
            # (the [128, 2, 1024] DoubleRow layout).
            cc_in = [dram.tile([512, 2048], F8, name=f"cc_in{x}")
                     for x in range(2)]
            cc_out = [dram.tile([1024, 2048], F8, name=f"cc_out{x}")
                      for x in range(2)]
            # S-sum exchange: each core only sums its own 1024 j rows; the
            # partner half arrives via a tiny pairwise AllReduce (fp32 add).
            s_in = dram.tile([2, D], F32, name="s_in")
            s_out = dram.tile([2, D], F32, name="s_out")

            # ---- phase A: kv projection -> fp8 staging -> pairwise AG ----
            with tc.tile_pool(name="phaseA", bufs=1) as pa:
                dataT = [pa.tile([128, TOK], BF16, name=f"dataT{k}",
                                 tag=f"dataT{k}") for k in range(KT)]
                wkv = [pa.tile([128, 2 * D], BF16, name=f"wkv{k}",
                               tag=f"wkv{k}") for k in range(KT)]
                wq = [pa.tile([128, D], BF16, name=f"wq{k}", tag=f"wq{k}")
                      for k in range(KT)]
                for k in range(KT):
                    nc.sync.dma_start(dataT[k][:], dataT_d[k * 128:(k + 1) * 128, :])
                    nc.sync.dma_start(wkv[k][:], wkv_d[k * 128:(k + 1) * 128, :])
                for k in range(KT):
                    nc.sync.dma_start(wq[k][:], wq_d[k * 128:(k + 1) * 128, :])
                for u in range(PAIRS):
                    nc.sync.dma_start(pbr[u][:], pbr_d[u * 128:(u + 1) * 128, :])
                for k in range(KT):
                    nc.sync.dma_start(wout[k][:], wout_d[k * 128:(k + 1) * 128, :])

                for m in range(NG):
                    ps = [psp.tile([128, 512], F32, name=f"ps{m}_{i}",
                                   tag="mm") for i in range(4)]
                    for k in range(KT):
                        for i in range(4):
                            nc.tensor.matmul(
                                ps[i][:], dataT[k][:, m * 128:(m + 1) * 128],
                                wkv[k][:, i * 512:(i + 1) * 512],
                                start=(k == 0),
                                stop=(k == KT - 1 and not with_qkv_bias),
                            )
                    if with_qkv_bias:
                        for i in range(4):
                            nc.tensor.matmul(
                                ps[i][:], ones1r[:], bkv[:, i * 512:(i + 1) * 512],
                                start=False, stop=True,
                            )
                    ek = pa.tile([128, D], BF16, name=f"ek{m}", tag="ek", bufs=3)
                    ekv = pa.tile([128, D], BF16, name=f"ekv{m}", tag="ekv",
                                  bufs=3)
                    for ih in range(2):
                        sl = slice(ih * 512, (ih + 1) * 512)
                        nc.scalar.activation(ek[:, sl], ps[ih][:], AF.Exp)
                        nc.vector.tensor_mul(ekv[:, sl], ek[:, sl], ps[2 + ih][:])
                        # S sums over this tile's 128 j rows (bf16 source,
                        # fp32 accum): row 0 <- ek, row 32 <- ekv
                        nc.tensor.matmul(
                            s2[ih][0:1, :], onescol[:], ek[:, sl],
                            start=(m == 0), stop=(m == NG - 1),
                            skip_group_check=True)
                        nc.tensor.matmul(
                            s2[ih][32:33, :], onescol[:], ekv[:, sl],
                            start=(m == 0), stop=(m == NG - 1),
                            skip_group_check=True)
                    ek8 = pa.tile([128, D], F8, name=f"ek8{m}", tag="ek8",
                                  bufs=3)
                    ekv8 = pa.tile([128, D], F8, name=f"ekv8{m}", tag="ekv8",
                                   bufs=3)
                    nc.vector.tensor_copy(ek8[:], ek[:])
                    nc.vector.tensor_copy(ekv8[:], ekv[:])
                    x, mm = m // 4, m % 4
                    q, t = mm // 2, mm % 2
                    nc.sync.dma_start(
                        cc_in[x][q * 128:(q + 1) * 128,
                                 t * 1024:(t + 1) * 1024], ek8[:])
                    nc.sync.dma_start(
                        cc_in[x][256 + q * 128:256 + (q + 1) * 128,
                                 t * 1024:(t + 1) * 1024], ekv8[:])
                    if m in (3, 7):
                        nc.gpsimd.collective_compute(
                            "AllGather", ALU.bypass,
                            replica_groups=PAIR_GROUPS,
                            ins=[cc_in[m // 4][:].opt()],
                            outs=[cc_out[m // 4][:].opt()],
                        )

                # S finalize: PSUM rows -> fp32 SBUF rows -> pairwise
                # AllReduce (adds the partner half of the j range) -> back
                # to SBUF -> per-partition columns via K=1 matmuls
                # (transpose of a [1,128] strip)
                for ih in range(2):
                    sl = slice(ih * 512, (ih + 1) * 512)
                    nc.scalar.copy(srow_d[0:1, sl], s2[ih][0:1, :])
                    nc.scalar.copy(srow_n[0:1, sl], s2[ih][32:33, :])
                nc.sync.dma_start(s_in[0:1, :], srow_d[:])
                nc.sync.dma_start(s_in[1:2, :], srow_n[:])
                nc.gpsimd.collective_compute(
                    "AllReduce", ALU.add,
                    replica_groups=PAIR_GROUPS,
                    ins=[s_in[:].opt()],
                    outs=[s_out[:].opt()],
                )
                sr2_d = pa.tile([1, D], F32, name="sr2_d", tag="sr2_d")
                sr2_n = pa.tile([1, D], F32, name="sr2_n", tag="sr2_n")
                nc.sync.dma_start(sr2_d[:], s_out[0:1, :])
                nc.sync.dma_start(sr2_n[:], s_out[1:2, :])
                stp = psp.tile([128, 512], F32, name="stp", tag="mm")
                for c in range(KT):
                    nc.tensor.matmul(
                        stp[:, c:c + 1],
                        sr2_d[0:1, c * 128:(c + 1) * 128], ones11[:],
                        skip_group_check=True)
                    nc.tensor.matmul(
                        stp[:, 8 + c:9 + c],
                        sr2_n[0:1, c * 128:(c + 1) * 128], ones11[:],
                        skip_group_check=True)
                nc.vector.tensor_copy(ST[:], stp[:, 0:16])

                # q^T projection + sigmoid (overlaps the collectives)
                for c in range(KT):
                    psq = [psp.tile([128, 512], F32, name=f"psq{c}_{ih}",
                                    tag="mm") for ih in range(2)]
                    for k in range(KT):
                        for ih in range(2):
                            nc.tensor.matmul(
                                psq[ih][:], wq[k][:, c * 128:(c + 1) * 128],
                                dataT[k][:, ih * 512:(ih + 1) * 512],
                                start=(k == 0), stop=(k == KT - 1),
                            )
                    for ih in range(2):
                        sl = slice(ih * 512, (ih + 1) * 512)
                        if with_qkv_bias:
                            nc.scalar.activation(
                                sq_t[c][:, sl], psq[ih][:], AF.Sigmoid,
                                bias=bqt[:, c:c + 1])
                        else:
                            nc.scalar.activation(
                                sq_t[c][:, sl], psq[ih][:], AF.Sigmoid)

            # ---- phase B: num/den DoubleRow accumulation + epilogue ----
            with tc.tile_pool(name="phaseB", bufs=1) as pb:
                ekg, ekvg = [], []
                for u in range(PAIRS):
                    x, hp, q = u // 4, (u // 2) % 2, u % 2
                    g = pb.tile([128, 2, TOK], F8, name=f"ekg{u}",
                                tag=f"ekg{u}")
                    nc.sync.dma_start(
                        g[:], cc_out[x][hp * 512 + q * 128:
                                        hp * 512 + (q + 1) * 128, :])
                    ekg.append(g)
                    gv = pb.tile([128, 2, TOK], F8, name=f"ekvg{u}",
                                 tag=f"ekvg{u}")
                    nc.sync.dma_start(
                        gv[:], cc_out[x][hp * 512 + 256 + q * 128:
                                         hp * 512 + 256 + (q + 1) * 128, :])
                    ekvg.append(gv)

                yT = [pb.tile([128, TOK], BF16, name=f"yT{c}", tag=f"yT{c}")
                      for c in range(KT)]

                for c in range(KT):
                    cs = slice(c * 128, (c + 1) * 128)
                    pn = [psp.tile([128, 512], F32, name=f"pn{c}_{ih}",
                                   tag="mm") for ih in range(2)]
                    pd = [psp.tile([128, 512], F32, name=f"pd{c}_{ih}",
                                   tag="mm") for ih in range(2)]
                    for u in range(PAIRS):
                        for ih in range(2):
                            isl = slice(ih * 512, (ih + 1) * 512)
                            nc.tensor.matmul(
                                pn[ih][:], ekvg[u][:, :, cs],
                                pbr[u][:, :, isl],
                                start=(u == 0), stop=(u == PAIRS - 1),
                                perf_mode=DR)
                        for ih in range(2):
                            isl = slice(ih * 512, (ih + 1) * 512)
                            nc.tensor.matmul(
                                pd[ih][:], ekg[u][:, :, cs],
                                pbr[u][:, :, isl],
                                start=(u == 0), stop=(u == PAIRS - 1),
                                perf_mode=DR)
                    for ih in range(2):
                        sl = slice(ih * 512, (ih + 1) * 512)
                        den = pb.tile([128, 512], F32, name=f"den{c}{ih}",
                                      tag="den", bufs=3)
                        nc.vector.tensor_scalar(
                            den[:], pd[ih][:], 1.0 / SCALE, ST[:, c:c + 1],
                            ALU.mult, ALU.add)
                        rec = pb.tile([128, 512], F32, name=f"rec{c}{ih}",
                                      tag="rec", bufs=3)
                        nc.vector.reciprocal_approx_fast(rec[:], den[:])
                        numf = pb.tile([128, 512], F32, name=f"numf{c}{ih}",
                                       tag="numf", bufs=3)
                        nc.scalar.activation(
                            numf[:], pn[ih][:], AF.Identity,
                            bias=ST[:, 8 + c:9 + c], scale=1.0 / SCALE)
                        tt = pb.tile([128, 512], F32, name=f"tt{c}{ih}",
                                     tag="tt", bufs=3)
                        nc.vector.tensor_mul(tt[:], numf[:], rec[:])
                        nc.vector.tensor_mul(yT[c][:, sl], tt[:],
                                             sq_t[c][:, sl])

                # output projection: lhsT = y^T directly
                for it in range(NG):
                    po = [psp.tile([128, 512], F32, name=f"po{it}_{n}",
                                   tag="mm") for n in range(2)]
                    for c in range(KT):
                        for n in range(2):
                            nc.tensor.matmul(
                                po[n][:], yT[c][:, it * 128:(it + 1) * 128],
                                wout[c][:, n * 512:(n + 1) * 512],
                                start=(c == 0),
                                stop=(c == KT - 1 and not with_out_bias))
                    if with_out_bias:
                        for n in range(2):
                            nc.tensor.matmul(
                                po[n][:], ones1r[:],
                                bout[:, n * 512:(n + 1) * 512],
                                start=False, stop=True)
                    for n in range(2):
                        osb = pb.tile([128, 512], F32, name=f"osb{it}_{n}",
                                      tag="osb", bufs=4)
                        nc.scalar.copy(osb[:], po[n][:])
                        nc.sync.dma_start(
                            out_d[it * 128:(it + 1) * 128,
                                  n * 512:(n + 1) * 512], osb[:])

    nc.compile()
    return nc


def _prep_inputs(data, W_qkv, b_qkv, pos_bias_param, W_out, b_out):
    bf = ml_dtypes.bfloat16
    f8 = ml_dtypes.float8_e4m3
    data = np.asarray(data, np.float32)
    W_qkv = np.asarray(W_qkv, np.float32)
    b_qkv = np.asarray(b_qkv, np.float32)
    pos_bias_param = np.asarray(pos_bias_param, np.float32)
    W_out = np.asarray(W_out, np.float32)
    b_out = np.asarray(b_out, np.float32)

    with_qkv_bias = bool(np.any(b_qkv))
    with_out_bias = bool(np.any(b_out))

    wq = np.ascontiguousarray(W_qkv[:, :D]).astype(bf)
    wkv = np.ascontiguousarray(W_qkv[:, D:]).astype(bf)
    wout = W_out.astype(bf)
    # pbr[j, i] = expm1(pb[i, j]) * SCALE, fp8 (correction term of exp(pb))
    pbr_full = np.clip(np.expm1(pos_bias_param.T) * SCALE, -240.0, 240.0)
    pbr_full = pbr_full.astype(f8)

    in_maps = []
    for r in range(N_CORES):
        b, h = r // 2, r % 2
        isl = slice(h * TOK, (h + 1) * TOK)
        dT = np.ascontiguousarray(data[isl, b, :].T).astype(bf)  # [d_in, tok]
        # pair-block layout: rows u*128.. hold j-pair u; column halves are
        # the two 128-j subtiles (DoubleRow [128, 2, 1024])
        pbr_c = np.empty((TOK, 2048), f8)
        for u in range(PAIRS):
            x, hp, q = u // 4, (u // 2) % 2, u % 2
            J0 = hp * 1024 + x * 512 + q * 256
            pbr_c[u * 128:(u + 1) * 128, :TOK] = pbr_full[J0:J0 + 128, isl]
            pbr_c[u * 128:(u + 1) * 128, TOK:] = pbr_full[J0 + 128:J0 + 256, isl]
        m = {"dataT": dT, "wq": wq, "wkv": wkv, "pbr": pbr_c, "wout": wout}
        if with_qkv_bias:
            m["bkv"] = np.ascontiguousarray(b_qkv[D:]).reshape(1, 2 * D).astype(bf)
            m["bqt"] = np.ascontiguousarray(
                b_qkv[:D].reshape(KT, 128).T).astype(np.float32)
        if with_out_bias:
            m["bout"] = b_out.reshape(1, D).astype(bf)
        in_maps.append(m)
    return in_maps, with_qkv_bias, with_out_bias


def run(data, W_qkv, b_qkv, pos_bias_param, W_out, b_out, **spmd_kwargs):
    in_maps, wb, ob = _prep_inputs(data, W_qkv, b_qkv, pos_bias_param, W_out,
                                   b_out)
    key = (wb, ob)
    if key not in _cache:
        _cache[key] = build(wb, ob)
    nc = _cache[key]
    res = run_bass_kernel_spmd(nc, in_maps, core_ids=list(range(N_CORES)),
                               **spmd_kwargs)
    out = np.empty((T, B, D), np.float32)
    for r in range(N_CORES):
        b, h = r // 2, r % 2
        out[h * TOK:(h + 1) * TOK, b, :] = res.results[r]["out"]
    return out, res


def kernel(data, W_qkv, b_qkv, pos_bias_param, W_out, b_out):
    out, _ = run(data, W_qkv, b_qkv, pos_bias_param, W_out, b_out)
    return out


# revision 6
# speedup vs baseline: 1.5825x; 1.0881x over previous
"""AFT (attention-free transformer) block on 8 TRN2 NeuronCores.

Reference computation (T=2048, B=4, D=1024):
    qkv = data @ W_qkv + b_qkv ; q,k,v = split(qkv)
    num = exp(pb - max_pb) @ (exp(k - max_k) * v)    (contraction over key pos j)
    den = exp(pb - max_pb) @ exp(k - max_k)
    out = (sigmoid(q) * num / den) @ W_out + b_out
The max shifts cancel exactly in num/den so the kernel drops them.

Sharding: hybrid (sequence-half x batch). Core r = 2b + h owns batch b and
query rows i in [h*1024, (h+1)*1024). Each core projects q/k/v for its own
1024 tokens; k-half exchange is a PAIRWISE AllGather (replica groups
[[0,1],[2,3],[4,5],[6,7]]) of fp8 exp(k) / exp(k)*v, two pipelined chunks.

Precision trick: exp(pb) = 1 + r with r = expm1(pb) in [-0.09, 0.1], so
    num = Snum + r @ ekv,   Snum[d] = sum_j ekv[j,d]   (i-independent)
    den = Sden + r @ ek
The S sums are computed in bf16/fp32 (ones-matmul on the PE, then a
K=1-matmul transpose into per-partition columns); the big TxT einsum runs
on the small correction term with BOTH operands fp8e4 and
perf_mode=DoubleRow (K virtualized to 256, ~2x PE throughput). The fp8
quantization error only touches the ~2% correction, keeping overall rel
err ~5e-3. r is pre-scaled by 64 on the host (epilogue rescales by 1/64).

Everything downstream of the pb einsum is TRANSPOSED ([d,i] layout): the q
projection emits sigmoid(q)^T directly (lhsT = W_q), num/den come out of
the DoubleRow matmuls as [d_chunk, i], and the output projection consumes
y^T as lhsT directly -- no PE transposes, no spill/merge passes.

Timeline per core: kv projection (8 token tiles, chunk AGs fired at tiles
3/7) -> S finalize -> q^T projection + sigmoid (covers AG wire time) ->
num/den DoubleRow accumulation (16 j-tiles as 8 pairs, single PSUM pass)
-> epilogue (reciprocal, sigmoid multiply) -> output projection.
"""

import numpy as np
import ml_dtypes

from concourse import bacc, bass, mybir, tile
from concourse.bass_utils import run_bass_kernel_spmd

BF16 = mybir.dt.bfloat16
F32 = mybir.dt.float32
F8 = mybir.dt.float8e4
AF = mybir.ActivationFunctionType
ALU = mybir.AluOpType
DR = mybir.MatmulPerfMode.DoubleRow

N_CORES = 8
T, B, D = 2048, 4, 1024
TOK = 1024                 # tokens per core: 1024 query rows of one batch
KT = D // 128              # 8 contraction tiles for d_in
NG = TOK // 128            # 8 token/query tile groups
PAIRS = T // 256           # 8 j-block pairs (DoubleRow processes 256 j rows)
SCALE = 64.0               # host pre-scale on expm1(pb) for fp8 range
PAIR_GROUPS = [[0, 1], [2, 3], [4, 5], [6, 7]]

_cache = {}


def build(with_qkv_bias: bool, with_out_bias: bool):
    nc = bacc.Bacc(None, target_bir_lowering=False)

    dataT_d = nc.dram_tensor("dataT", [D, TOK], BF16, kind="ExternalInput")
    wkv_d = nc.dram_tensor("wkv", [D, 2 * D], BF16, kind="ExternalInput")
    wq_d = nc.dram_tensor("wq", [D, D], BF16, kind="ExternalInput")
    pbr_d = nc.dram_tensor("pbr", [TOK, 2048], F8, kind="ExternalInput")
    wout_d = nc.dram_tensor("wout", [D, D], BF16, kind="ExternalInput")
    out_d = nc.dram_tensor("out", [TOK, D], F32, kind="ExternalOutput")
    if with_qkv_bias:
        bkv_d = nc.dram_tensor("bkv", [1, 2 * D], BF16, kind="ExternalInput")
        bqt_d = nc.dram_tensor("bqt", [128, KT], F32, kind="ExternalInput")
    if with_out_bias:
        bout_d = nc.dram_tensor("bout", [1, D], BF16, kind="ExternalInput")

    with tile.TileContext(nc) as tc:
        with (
            tc.tile_pool(name="persist", bufs=1) as pp,
            tc.tile_pool(name="psum", bufs=6, space="PSUM") as psp,
            tc.tile_pool(name="psum_s", bufs=1, space="PSUM") as pss,
            tc.tile_pool(name="dram", bufs=1, space="DRAM") as dram,
        ):
            # ---- persistent SBUF tensors ----
            onescol = pp.tile([128, 1], BF16, name="onescol", tag="onescol")
            nc.gpsimd.memset(onescol[:], 1.0)
            ones11 = pp.tile([1, 1], F32, name="ones11", tag="ones11")
            nc.gpsimd.memset(ones11[:], 1.0)
            # ST cols 0-7: Sden per d-chunk; cols 8-15: Snum
            ST = pp.tile([128, 16], F32, name="ST", tag="ST")
            srow_d = pp.tile([1, D], F32, name="srow_d", tag="srow_d")
            srow_n = pp.tile([1, D], F32, name="srow_n", tag="srow_n")
            wout = [pp.tile([128, D], BF16, name=f"wout{k}", tag=f"wout{k}")
                    for k in range(KT)]
            pbr = [pp.tile([128, 2, TOK], F8, name=f"pbr{u}", tag=f"pbr{u}")
                   for u in range(PAIRS)]
            sq_t = [pp.tile([128, TOK], BF16, name=f"sq{c}", tag=f"sq{c}")
                    for c in range(KT)]
            if with_qkv_bias or with_out_bias:
                ones1r = pp.tile([1, 128], BF16, name="ones1r", tag="ones1r")
                nc.gpsimd.memset(ones1r[:], 1.0)
            if with_qkv_bias:
                bkv = pp.tile([1, 2 * D], BF16, name="bkv", tag="bkv")
                nc.sync.dma_start(bkv[:], bkv_d[:])
                bqt = pp.tile([128, KT], F32, name="bqt", tag="bqt")
                nc.sync.dma_start(bqt[:], bqt_d[:])
            if with_out_bias:
                bout = pp.tile([1, D], BF16, name="bout", tag="bout")
                nc.sync.dma_start(bout[:], bout_d[:])

            # S accumulators: row 0 = Sden, row 32 = Snum (PE col-strips)
            s2 = [pss.tile([64, 512], F32, name=f"s2_{ih}", tag=f"s2_{ih}")
                  for ih in range(2)]

            # collective bounce buffers, fp8, one chunk per 512 own tokens:
            # rows 0:256 = ek pairs (q=0,1), 256:512 = ekv pairs; within a
            # pair row-block the two 128-j subtiles sit in column halves
            # (the [128, 2, 1024] DoubleRow layout).
            cc_in = [dram.tile([512, 2048], F8, name=f"cc_in{x}")
                     for x in range(2)]
            cc_out = [dram.tile([1024, 2048], F8, name=f"cc_out{x}")
                      for x in range(2)]
            # S-sum exchange: each core only sums its own 1024 j rows; the
            # partner half arrives via a tiny pairwise AllReduce (fp32 add).
            s_in = dram.tile([2, D], F32, name="s_in")
            s_out = dram.tile([2, D], F32, name="s_out")

            # ---- phase A: kv projection -> fp8 staging -> pairwise AG ----
            with tc.tile_pool(name="phaseA", bufs=1) as pa:
                dataT = [pa.tile([128, TOK], BF16, name=f"dataT{k}",
                                 tag=f"dataT{k}") for k in range(KT)]
                wkv = [pa.tile([128, 2 * D], BF16, name=f"wkv{k}",
                               tag=f"wkv{k}") for k in range(KT)]
                wq = [pa.tile([128, D], BF16, name=f"wq{k}", tag=f"wq{k}")
                      for k in range(KT)]
                for k in range(KT):
                    nc.sync.dma_start(dataT[k][:], dataT_d[k * 128:(k + 1) * 128, :])
                    nc.sync.dma_start(wkv[k][:], wkv_d[k * 128:(k + 1) * 128, :])
                for k in range(KT):
                    nc.sync.dma_start(wq[k][:], wq_d[k * 128:(k + 1) * 128, :])
                for u in range(PAIRS):
                    nc.sync.dma_start(pbr[u][:], pbr_d[u * 128:(u + 1) * 128, :])
                for k in range(KT):
                    nc.sync.dma_start(wout[k][:], wout_d[k * 128:(k + 1) * 128, :])

                for m in range(NG):
                    ps = [psp.tile([128, 512], F32, name=f"ps{m}_{i}",
                                   tag="mm") for i in range(4)]
                    for k in range(KT):
                        for i in range(4):
                            nc.tensor.matmul(
                                ps[i][:], dataT[k][:, m * 128:(m + 1) * 128],
                                wkv[k][:, i * 512:(i + 1) * 512],
                                start=(k == 0),
                                stop=(k == KT - 1 and not with_qkv_bias),
                            )
                    if with_qkv_bias:
                        for i in range(4):
                            nc.tensor.matmul(
                                ps[i][:], ones1r[:], bkv[:, i * 512:(i + 1) * 512],
                                start=False, stop=True,
                            )
                    ek = pa.tile([128, D], BF16, name=f"ek{m}", tag="ek", bufs=3)
                    ekv = pa.tile([128, D], BF16, name=f"ekv{m}", tag="ekv",
                                  bufs=3)
                    for ih in range(2):
                        sl = slice(ih * 512, (ih + 1) * 512)
                        nc.scalar.activation(ek[:, sl], ps[ih][:], AF.Exp)
                        nc.vector.tensor_mul(ekv[:, sl], ek[:, sl], ps[2 + ih][:])
                        # S sums over this tile's 128 j rows (bf16 source,
                        # fp32 accum): row 0 <- ek, row 32 <- ekv
                        nc.tensor.matmul(
                            s2[ih][0:1, :], onescol[:], ek[:, sl],
                            start=(m == 0), stop=(m == NG - 1),
                            skip_group_check=True)
                        nc.tensor.matmul(
                            s2[ih][32:33, :], onescol[:], ekv[:, sl],
                            start=(m == 0), stop=(m == NG - 1),
                            skip_group_check=True)
                    ek8 = pa.tile([128, D], F8, name=f"ek8{m}", tag="ek8",
                                  bufs=3)
                    ekv8 = pa.tile([128, D], F8, name=f"ekv8{m}", tag="ekv8",
                                   bufs=3)
                    nc.vector.tensor_copy(ek8[:], ek[:])
                    nc.vector.tensor_copy(ekv8[:], ekv[:])
                    x, mm = m // 4, m % 4
                    q, t = mm // 2, mm % 2
                    nc.sync.dma_start(
                        cc_in[x][q * 128:(q + 1) * 128,
                                 t * 1024:(t + 1) * 1024], ek8[:])
                    nc.sync.dma_start(
                        cc_in[x][256 + q * 128:256 + (q + 1) * 128,
                                 t * 1024:(t + 1) * 1024], ekv8[:])
                    if m in (3, 7):
                        nc.gpsimd.collective_compute(
                            "AllGather", ALU.bypass,
                            replica_groups=PAIR_GROUPS,
                            ins=[cc_in[m // 4][:].opt()],
                            outs=[cc_out[m // 4][:].opt()],
                        )

                # S finalize: PSUM rows -> fp32 SBUF rows -> pairwise
                # AllReduce (adds the partner half of the j range) -> back
                # to SBUF -> per-partition columns via K=1 matmuls
                # (transpose of a [1,128] strip)
                for ih in range(2):
                    sl = slice(ih * 512, (ih + 1) * 512)
                    nc.scalar.copy(srow_d[0:1, sl], s2[ih][0:1, :])
                    nc.scalar.copy(srow_n[0:1, sl], s2[ih][32:33, :])
                nc.sync.dma_start(s_in[0:1, :], srow_d[:])
                nc.sync.dma_start(s_in[1:2, :], srow_n[:])
                nc.gpsimd.collective_compute(
                    "AllReduce", ALU.add,
                    replica_groups=PAIR_GROUPS,
                    ins=[s_in[:].opt()],
                    outs=[s_out[:].opt()],
                )
                sr2_d = pa.tile([1, D], F32, name="sr2_d", tag="sr2_d")
                sr2_n = pa.tile([1, D], F32, name="sr2_n", tag="sr2_n")
                nc.sync.dma_start(sr2_d[:], s_out[0:1, :])
                nc.sync.dma_start(sr2_n[:], s_out[1:2, :])

                # q^T projection + sigmoid (overlaps the collectives)
                for c in range(KT):
                    psq = [psp.tile([128, 512], F32, name=f"psq{c}_{ih}",
                                    tag="mm") for ih in range(2)]
                    for k in range(KT):
                        for ih in range(2):
                            nc.tensor.matmul(
                                psq[ih][:], wq[k][:, c * 128:(c + 1) * 128],
                                dataT[k][:, ih * 512:(ih + 1) * 512],
                                start=(k == 0), stop=(k == KT - 1),
                            )
                    for ih in range(2):
                        sl = slice(ih * 512, (ih + 1) * 512)
                        if with_qkv_bias:
                            nc.scalar.activation(
                                sq_t[c][:, sl], psq[ih][:], AF.Sigmoid,
                                bias=bqt[:, c:c + 1])
                        else:
                            nc.scalar.activation(
                                sq_t[c][:, sl], psq[ih][:], AF.Sigmoid)

                # S transpose AFTER the q projection: these 16 micro-MMs
                # wait on the AllReduce, so they must sit behind the q MMs
                # in the in-order PE queue (not ahead of them).
                stp = psp.tile([128, 512], F32, name="stp", tag="mm")
                for c in range(KT):
                    nc.tensor.matmul(
                        stp[:, c:c + 1],
                        sr2_d[0:1, c * 128:(c + 1) * 128], ones11[:],
                        skip_group_check=True)
                    nc.tensor.matmul(
                        stp[:, 8 + c:9 + c],
                        sr2_n[0:1, c * 128:(c + 1) * 128], ones11[:],
                        skip_group_check=True)
                nc.vector.tensor_copy(ST[:], stp[:, 0:16])

            # ---- phase B: num/den DoubleRow accumulation + epilogue ----
            with tc.tile_pool(name="phaseB", bufs=1) as pb:
                ekg, ekvg = [], []
                for u in range(PAIRS):
                    x, hp, q = u // 4, (u // 2) % 2, u % 2
                    g = pb.tile([128, 2, TOK], F8, name=f"ekg{u}",
                                tag=f"ekg{u}")
                    nc.sync.dma_start(
                        g[:], cc_out[x][hp * 512 + q * 128:
                                        hp * 512 + (q + 1) * 128, :])
                    ekg.append(g)
                    gv = pb.tile([128, 2, TOK], F8, name=f"ekvg{u}",
                                 tag=f"ekvg{u}")
                    nc.sync.dma_start(
                        gv[:], cc_out[x][hp * 512 + 256 + q * 128:
                                         hp * 512 + 256 + (q + 1) * 128, :])
                    ekvg.append(gv)

                yT = [pb.tile([128, TOK], BF16, name=f"yT{c}", tag=f"yT{c}")
                      for c in range(KT)]

                for c in range(KT):
                    cs = slice(c * 128, (c + 1) * 128)
                    pn = [psp.tile([128, 512], F32, name=f"pn{c}_{ih}",
                                   tag="mm") for ih in range(2)]
                    pd = [psp.tile([128, 512], F32, name=f"pd{c}_{ih}",
                                   tag="mm") for ih in range(2)]
                    for u in range(PAIRS):
                        for ih in range(2):
                            isl = slice(ih * 512, (ih + 1) * 512)
                            nc.tensor.matmul(
                                pn[ih][:], ekvg[u][:, :, cs],
                                pbr[u][:, :, isl],
                                start=(u == 0), stop=(u == PAIRS - 1),
                                perf_mode=DR)
                        for ih in range(2):
                            isl = slice(ih * 512, (ih + 1) * 512)
                            nc.tensor.matmul(
                                pd[ih][:], ekg[u][:, :, cs],
                                pbr[u][:, :, isl],
                                start=(u == 0), stop=(u == PAIRS - 1),
                                perf_mode=DR)
                    for ih in range(2):
                        sl = slice(ih * 512, (ih + 1) * 512)
                        den = pb.tile([128, 512], F32, name=f"den{c}{ih}",
                                      tag="den", bufs=3)
                        nc.vector.tensor_scalar(
                            den[:], pd[ih][:], 1.0 / SCALE, ST[:, c:c + 1],
                            ALU.mult, ALU.add)
                        rec = pb.tile([128, 512], F32, name=f"rec{c}{ih}",
                                      tag="rec", bufs=3)
                        nc.vector.reciprocal_approx_fast(rec[:], den[:])
                        numf = pb.tile([128, 512], F32, name=f"numf{c}{ih}",
                                       tag="numf", bufs=3)
                        nc.scalar.activation(
                            numf[:], pn[ih][:], AF.Identity,
                            bias=ST[:, 8 + c:9 + c], scale=1.0 / SCALE)
                        tt = pb.tile([128, 512], F32, name=f"tt{c}{ih}",
                                     tag="tt", bufs=3)
                        nc.vector.tensor_mul(tt[:], numf[:], rec[:])
                        nc.vector.tensor_mul(yT[c][:, sl], tt[:],
                                             sq_t[c][:, sl])

                # output projection: lhsT = y^T directly
                for it in range(NG):
                    po = [psp.tile([128, 512], F32, name=f"po{it}_{n}",
                                   tag="mm") for n in range(2)]
                    for c in range(KT):
                        for n in range(2):
                            nc.tensor.matmul(
                                po[n][:], yT[c][:, it * 128:(it + 1) * 128],
                                wout[c][:, n * 512:(n + 1) * 512],
                                start=(c == 0),
                                stop=(c == KT - 1 and not with_out_bias))
                    if with_out_bias:
                        for n in range(2):
                            nc.tensor.matmul(
                                po[n][:], ones1r[:],
                                bout[:, n * 512:(n + 1) * 512],
                                start=False, stop=True)
                    for n in range(2):
                        osb = pb.tile([128, 512], F32, name=f"osb{it}_{n}",
                                      tag="osb", bufs=4)
                        nc.scalar.copy(osb[:], po[n][:])
                        nc.sync.dma_start(
                            out_d[it * 128:(it + 1) * 128,
                                  n * 512:(n + 1) * 512], osb[:])

    nc.compile()
    return nc


def _prep_inputs(data, W_qkv, b_qkv, pos_bias_param, W_out, b_out):
    bf = ml_dtypes.bfloat16
    f8 = ml_dtypes.float8_e4m3
    data = np.asarray(data, np.float32)
    W_qkv = np.asarray(W_qkv, np.float32)
    b_qkv = np.asarray(b_qkv, np.float32)
    pos_bias_param = np.asarray(pos_bias_param, np.float32)
    W_out = np.asarray(W_out, np.float32)
    b_out = np.asarray(b_out, np.float32)

    with_qkv_bias = bool(np.any(b_qkv))
    with_out_bias = bool(np.any(b_out))

    wq = np.ascontiguousarray(W_qkv[:, :D]).astype(bf)
    wkv = np.ascontiguousarray(W_qkv[:, D:]).astype(bf)
    wout = W_out.astype(bf)
    # pbr[j, i] = expm1(pb[i, j]) * SCALE, fp8 (correction term of exp(pb))
    pbr_full = np.clip(np.expm1(pos_bias_param.T) * SCALE, -240.0, 240.0)
    pbr_full = pbr_full.astype(f8)

    in_maps = []
    for r in range(N_CORES):
        b, h = r // 2, r % 2
        isl = slice(h * TOK, (h + 1) * TOK)
        dT = np.ascontiguousarray(data[isl, b, :].T).astype(bf)  # [d_in, tok]
        # pair-block layout: rows u*128.. hold j-pair u; column halves are
        # the two 128-j subtiles (DoubleRow [128, 2, 1024])
        pbr_c = np.empty((TOK, 2048), f8)
        for u in range(PAIRS):
            x, hp, q = u // 4, (u // 2) % 2, u % 2
            J0 = hp * 1024 + x * 512 + q * 256
            pbr_c[u * 128:(u + 1) * 128, :TOK] = pbr_full[J0:J0 + 128, isl]
            pbr_c[u * 128:(u + 1) * 128, TOK:] = pbr_full[J0 + 128:J0 + 256, isl]
        m = {"dataT": dT, "wq": wq, "wkv": wkv, "pbr": pbr_c, "wout": wout}
        if with_qkv_bias:
            m["bkv"] = np.ascontiguousarray(b_qkv[D:]).reshape(1, 2 * D).astype(bf)
            m["bqt"] = np.ascontiguousarray(
                b_qkv[:D].reshape(KT, 128).T).astype(np.float32)
        if with_out_bias:
            m["bout"] = b_out.reshape(1, D).astype(bf)
        in_maps.append(m)
    return in_maps, with_qkv_bias, with_out_bias


def run(data, W_qkv, b_qkv, pos_bias_param, W_out, b_out, **spmd_kwargs):
    in_maps, wb, ob = _prep_inputs(data, W_qkv, b_qkv, pos_bias_param, W_out,
                                   b_out)
    key = (wb, ob)
    if key not in _cache:
        _cache[key] = build(wb, ob)
    nc = _cache[key]
    res = run_bass_kernel_spmd(nc, in_maps, core_ids=list(range(N_CORES)),
                               **spmd_kwargs)
    out = np.empty((T, B, D), np.float32)
    for r in range(N_CORES):
        b, h = r // 2, r % 2
        out[h * TOK:(h + 1) * TOK, b, :] = res.results[r]["out"]
    return out, res


def kernel(data, W_qkv, b_qkv, pos_bias_param, W_out, b_out):
    out, _ = run(data, W_qkv, b_qkv, pos_bias_param, W_out, b_out)
    return out


# revision 17
# speedup vs baseline: 1.6819x; 1.0628x over previous
"""AFT (attention-free transformer) block on 8 TRN2 NeuronCores.

Reference computation (T=2048, B=4, D=1024):
    qkv = data @ W_qkv + b_qkv ; q,k,v = split(qkv)
    num = exp(pb - max_pb) @ (exp(k - max_k) * v)    (contraction over key pos j)
    den = exp(pb - max_pb) @ exp(k - max_k)
    out = (sigmoid(q) * num / den) @ W_out + b_out
The max shifts cancel exactly in num/den so the kernel drops them.

Sharding: hybrid (sequence-half x batch). Core r = 2b + h owns batch b and
query rows i in [h*1024, (h+1)*1024). Each core projects q/k/v for its own
1024 tokens; k-half exchange is a PAIRWISE AllGather (replica groups
[[0,1],[2,3],[4,5],[6,7]]) of fp8 exp(k) / exp(k)*v, two pipelined chunks.

Precision trick: exp(pb) = 1 + r with r = expm1(pb) in [-0.09, 0.1], so
    num = Snum + r @ ekv,   Snum[d] = sum_j ekv[j,d]   (i-independent)
    den = Sden + r @ ek
The S sums are computed in bf16/fp32 (ones-matmul on the PE, then a
K=1-matmul transpose into per-partition columns); the big TxT einsum runs
on the small correction term with BOTH operands fp8e4 and
perf_mode=DoubleRow (K virtualized to 256, ~2x PE throughput). The fp8
quantization error only touches the ~2% correction, keeping overall rel
err ~5e-3. r is pre-scaled by 64 on the host (epilogue rescales by 1/64).

Everything downstream of the pb einsum is TRANSPOSED ([d,i] layout): the q
projection emits sigmoid(q)^T directly (lhsT = W_q), num/den come out of
the DoubleRow matmuls as [d_chunk, i], and the output projection consumes
y^T as lhsT directly -- no PE transposes, no spill/merge passes.

Timeline per core: kv projection (8 token tiles, chunk AGs fired at tiles
3/7) -> S finalize -> q^T projection + sigmoid (covers AG wire time) ->
num/den DoubleRow accumulation (16 j-tiles as 8 pairs, single PSUM pass)
-> epilogue (reciprocal, sigmoid multiply) -> output projection.
"""

import numpy as np
import ml_dtypes

from concourse import bacc, bass, mybir, tile
from concourse.bass_utils import run_bass_kernel_spmd

BF16 = mybir.dt.bfloat16
F32 = mybir.dt.float32
F8 = mybir.dt.float8e4
AF = mybir.ActivationFunctionType
ALU = mybir.AluOpType
DR = mybir.MatmulPerfMode.DoubleRow

N_CORES = 8
T, B, D = 2048, 4, 1024
TOK = 1024                 # tokens per core: 1024 query rows of one batch
KT = D // 128              # 8 contraction tiles for d_in
NG = TOK // 128            # 8 token/query tile groups
PAIRS = T // 256           # 8 j-block pairs (DoubleRow processes 256 j rows)
SCALE = 64.0               # host pre-scale on expm1(pb) for fp8 range
PAIR_GROUPS = [[0, 1], [2, 3], [4, 5], [6, 7]]

_cache = {}


def build(with_qkv_bias: bool, with_out_bias: bool):
    nc = bacc.Bacc(None, target_bir_lowering=False)

    dataT_d = nc.dram_tensor("dataT", [D, TOK], BF16, kind="ExternalInput")
    wkv_d = nc.dram_tensor("wkv", [D, 2 * D], BF16, kind="ExternalInput")
    wq_d = nc.dram_tensor("wq", [D, D], BF16, kind="ExternalInput")
    pbr_d = nc.dram_tensor("pbr", [TOK, 2048], F8, kind="ExternalInput")
    wout_d = nc.dram_tensor("wout", [D, D], BF16, kind="ExternalInput")
    out_d = nc.dram_tensor("out", [TOK, D], F32, kind="ExternalOutput")
    if with_qkv_bias:
        bkv_d = nc.dram_tensor("bkv", [1, 2 * D], BF16, kind="ExternalInput")
        bqt_d = nc.dram_tensor("bqt", [128, KT], F32, kind="ExternalInput")
    if with_out_bias:
        bout_d = nc.dram_tensor("bout", [1, D], BF16, kind="ExternalInput")

    with tile.TileContext(nc) as tc:
        with (
            tc.tile_pool(name="persist", bufs=1) as pp,
            tc.tile_pool(name="psum", bufs=6, space="PSUM") as psp,
            tc.tile_pool(name="psum_s", bufs=1, space="PSUM") as pss,
            tc.tile_pool(name="dram", bufs=1, space="DRAM") as dram,
        ):
            # ---- persistent SBUF tensors ----
            onescol = pp.tile([128, 1], BF16, name="onescol", tag="onescol")
            nc.gpsimd.memset(onescol[:], 1.0)
            # 64.0 as the K=1 matmul rhs: ST holds 64*S so the 1/SCALE on
            # the einsum and the S bias cancel exactly in num/den
            c64 = pp.tile([1, 1], F32, name="c64", tag="c64")
            nc.gpsimd.memset(c64[:], SCALE)
            # ST cols 0-7: Sden per d-chunk; cols 8-15: Snum
            ST = pp.tile([128, 16], F32, name="ST", tag="ST")
            srow_d = pp.tile([1, D], F32, name="srow_d", tag="srow_d")
            srow_n = pp.tile([1, D], F32, name="srow_n", tag="srow_n")
            sr2_d = pp.tile([1, D], F32, name="sr2_d", tag="sr2_d")
            sr2_n = pp.tile([1, D], F32, name="sr2_n", tag="sr2_n")
            wout = [pp.tile([128, D], BF16, name=f"wout{k}", tag=f"wout{k}")
                    for k in range(KT)]
            pbr = [pp.tile([128, 2, TOK], F8, name=f"pbr{u}", tag=f"pbr{u}")
                   for u in range(PAIRS)]
            sq_t = [pp.tile([128, TOK], BF16, name=f"sq{c}", tag=f"sq{c}")
                    for c in range(KT)]
            if with_qkv_bias or with_out_bias:
                ones1r = pp.tile([1, 128], BF16, name="ones1r", tag="ones1r")
                nc.gpsimd.memset(ones1r[:], 1.0)
            if with_qkv_bias:
                bkv = pp.tile([1, 2 * D], BF16, name="bkv", tag="bkv")
                nc.sync.dma_start(bkv[:], bkv_d[:])
                bqt = pp.tile([128, KT], F32, name="bqt", tag="bqt")
                nc.sync.dma_start(bqt[:], bqt_d[:])
            if with_out_bias:
                bout = pp.tile([1, D], BF16, name="bout", tag="bout")
                nc.sync.dma_start(bout[:], bout_d[:])

            # S accumulators: row 0 = Sden, row 32 = Snum (PE col-strips)
            s2 = [pss.tile([64, 512], F32, name=f"s2_{ih}", tag=f"s2_{ih}")
                  for ih in range(2)]

            # collective bounce buffers, fp8, one chunk per 512 own tokens:
            # rows 0:256 = ek pairs (q=0,1), 256:512 = ekv pairs; within a
            # pair row-block the two 128-j subtiles sit in column halves
            # (the [128, 2, 1024] DoubleRow layout).
            # chunk 1 carries 4 extra rows: the core's own-half S sums as
            # raw fp32 bytes (rows 512-513 = Sden, 514-515 = Snum); both
            # halves' rows come back with the gather and are added on-chip,
            # so no separate AllReduce is needed.
            cc_in = [dram.tile([512 + 4 * x, 2048], F8, name=f"cc_in{x}")
                     for x in range(2)]
            cc_out = [dram.tile([1024 + 8 * x, 2048], F8, name=f"cc_out{x}")
                      for x in range(2)]

            # ---- phase A: kv projection -> fp8 staging -> pairwise AG ----
            with tc.tile_pool(name="phaseA", bufs=1) as pa:
                dataT = [pa.tile([128, TOK], BF16, name=f"dataT{k}",
                                 tag=f"dataT{k}") for k in range(KT)]
                wkv = [pa.tile([128, 2 * D], BF16, name=f"wkv{k}",
                               tag=f"wkv{k}") for k in range(KT)]
                wq = [pa.tile([128, D], BF16, name=f"wq{k}", tag=f"wq{k}")
                      for k in range(KT)]
                for k in range(KT):
                    nc.sync.dma_start(dataT[k][:], dataT_d[k * 128:(k + 1) * 128, :])
                    nc.sync.dma_start(wkv[k][:, :D],
                                      wkv_d[k * 128:(k + 1) * 128, :D])
                    nc.sync.dma_start(wkv[k][:, D:],
                                      wkv_d[k * 128:(k + 1) * 128, D:])
                for k in range(KT):
                    nc.sync.dma_start(wq[k][:], wq_d[k * 128:(k + 1) * 128, :])
                for u in range(PAIRS):
                    nc.sync.dma_start(pbr[u][:], pbr_d[u * 128:(u + 1) * 128, :])
                for k in range(KT):
                    nc.sync.dma_start(wout[k][:], wout_d[k * 128:(k + 1) * 128, :])

                for m in range(NG):
                    ps = [psp.tile([128, 512], F32, name=f"ps{m}_{i}",
                                   tag="mm") for i in range(4)]
                    for k in range(KT):
                        for i in range(4):
                            nc.tensor.matmul(
                                ps[i][:], dataT[k][:, m * 128:(m + 1) * 128],
                                wkv[k][:, i * 512:(i + 1) * 512],
                                start=(k == 0),
                                stop=(k == KT - 1 and not with_qkv_bias),
                            )
                    if with_qkv_bias:
                        for i in range(4):
                            nc.tensor.matmul(
                                ps[i][:], ones1r[:], bkv[:, i * 512:(i + 1) * 512],
                                start=False, stop=True,
                            )
                    ek = pa.tile([128, D], BF16, name=f"ek{m}", tag="ek", bufs=3)
                    ekv = pa.tile([128, D], BF16, name=f"ekv{m}", tag="ekv",
                                  bufs=3)
                    for ih in range(2):
                        sl = slice(ih * 512, (ih + 1) * 512)
                        nc.scalar.activation(ek[:, sl], ps[ih][:], AF.Exp)
                        nc.vector.tensor_mul(ekv[:, sl], ek[:, sl], ps[2 + ih][:])
                        # S sums over this tile's 128 j rows (bf16 source,
                        # fp32 accum): row 0 <- ek, row 32 <- ekv
                        nc.tensor.matmul(
                            s2[ih][0:1, :], onescol[:], ek[:, sl],
                            start=(m == 0), stop=(m == NG - 1),
                            skip_group_check=True)
                        nc.tensor.matmul(
                            s2[ih][32:33, :], onescol[:], ekv[:, sl],
                            start=(m == 0), stop=(m == NG - 1),
                            skip_group_check=True)
                    ek8 = pa.tile([128, D], F8, name=f"ek8{m}", tag="ek8",
                                  bufs=3)
                    ekv8 = pa.tile([128, D], F8, name=f"ekv8{m}", tag="ekv8",
                                   bufs=3)
                    nc.vector.tensor_copy(ek8[:], ek[:])
                    nc.vector.tensor_copy(ekv8[:], ekv[:])
                    x, mm = m // 4, m % 4
                    q, t = mm // 2, mm % 2
                    nc.sync.dma_start(
                        cc_in[x][q * 128:(q + 1) * 128,
                                 t * 1024:(t + 1) * 1024], ek8[:])
                    nc.sync.dma_start(
                        cc_in[x][256 + q * 128:256 + (q + 1) * 128,
                                 t * 1024:(t + 1) * 1024], ekv8[:])
                    if m == 3:
                        nc.gpsimd.collective_compute(
                            "AllGather", ALU.bypass,
                            replica_groups=PAIR_GROUPS,
                            ins=[cc_in[0][:].opt()],
                            outs=[cc_out[0][:].opt()],
                        )

                # S finalize: PSUM rows -> fp32 SBUF rows -> packed as raw
                # bytes into cc_in[1] rows 512-515, shipped by AG chunk 1
                for ih in range(2):
                    sl = slice(ih * 512, (ih + 1) * 512)
                    nc.scalar.copy(srow_d[0:1, sl], s2[ih][0:1, :])
                    nc.scalar.copy(srow_n[0:1, sl], s2[ih][32:33, :])
                for ih in range(2):
                    sl = slice(ih * 512, (ih + 1) * 512)
                    nc.sync.dma_start(cc_in[1][512 + ih:513 + ih, :],
                                      srow_d[:, sl].bitcast(F8))
                    nc.sync.dma_start(cc_in[1][514 + ih:515 + ih, :],
                                      srow_n[:, sl].bitcast(F8))
                nc.gpsimd.collective_compute(
                    "AllGather", ALU.bypass,
                    replica_groups=PAIR_GROUPS,
                    ins=[cc_in[1][:].opt()],
                    outs=[cc_out[1][:].opt()],
                )

                # q^T projection + sigmoid (overlaps the collectives)
                for c in range(KT):
                    psq = [psp.tile([128, 512], F32, name=f"psq{c}_{ih}",
                                    tag="mm") for ih in range(2)]
                    for k in range(KT):
                        for ih in range(2):
                            nc.tensor.matmul(
                                psq[ih][:], wq[k][:, c * 128:(c + 1) * 128],
                                dataT[k][:, ih * 512:(ih + 1) * 512],
                                start=(k == 0), stop=(k == KT - 1),
                            )
                    for ih in range(2):
                        sl = slice(ih * 512, (ih + 1) * 512)
                        if with_qkv_bias:
                            nc.scalar.activation(
                                sq_t[c][:, sl], psq[ih][:], AF.Sigmoid,
                                bias=bqt[:, c:c + 1])
                        else:
                            nc.scalar.activation(
                                sq_t[c][:, sl], psq[ih][:], AF.Sigmoid)

                def emit_s_transpose():
                    # 16 micro-MMs turning the summed S rows into
                    # per-partition bias columns, scaled by 64 (rhs = c64).
                    # They wait on AG chunk 1, so they are emitted BEHIND
                    # the first num/den MM block in the in-order PE queue
                    # (the MMs don't need ST; only the epilogue does).
                    stp = psp.tile([128, 512], F32, name="stp", tag="mm")
                    for c in range(KT):
                        nc.tensor.matmul(
                            stp[:, c:c + 1],
                            sr2_d[0:1, c * 128:(c + 1) * 128], c64[:],
                            skip_group_check=True)
                        nc.tensor.matmul(
                            stp[:, 8 + c:9 + c],
                            sr2_n[0:1, c * 128:(c + 1) * 128], c64[:],
                            skip_group_check=True)
                    nc.vector.tensor_copy(ST[:], stp[:, 0:16])

            # ---- phase B: num/den DoubleRow accumulation + epilogue ----
            with tc.tile_pool(name="phaseB", bufs=1) as pb:
                ekg, ekvg = [], []
                for u in range(PAIRS):
                    x, hp, q = u // 4, (u // 2) % 2, u % 2
                    rb = hp * (512 + 4 * x)  # rank base (chunk 1 rows: 516)
                    g = pb.tile([128, 2, TOK], F8, name=f"ekg{u}",
                                tag=f"ekg{u}")
                    nc.sync.dma_start(
                        g[:], cc_out[x][rb + q * 128:rb + (q + 1) * 128, :])
                    ekg.append(g)
                    gv = pb.tile([128, 2, TOK], F8, name=f"ekvg{u}",
                                 tag=f"ekvg{u}")
                    nc.sync.dma_start(
                        gv[:], cc_out[x][rb + 256 + q * 128:
                                         rb + 256 + (q + 1) * 128, :])
                    ekvg.append(gv)

                # S rows of both halves (raw fp32 bytes at rows 512-515 of
                # each rank block in cc_out[1]) -> SBUF -> add
                gs = [pb.tile([1, D], F32, name=f"gs{i}", tag=f"gs{i}")
                      for i in range(4)]  # [d0, n0, d1, n1]
                for rk in range(2):
                    rb = rk * 516
                    for ih in range(2):
                        sl = slice(ih * 512, (ih + 1) * 512)
                        nc.sync.dma_start(
                            gs[2 * rk][:, sl].bitcast(F8),
                            cc_out[1][rb + 512 + ih:rb + 513 + ih, :])
                        nc.sync.dma_start(
                            gs[2 * rk + 1][:, sl].bitcast(F8),
                            cc_out[1][rb + 514 + ih:rb + 515 + ih, :])
                nc.vector.tensor_add(sr2_d[:], gs[0][:], gs[2][:])
                nc.vector.tensor_add(sr2_n[:], gs[1][:], gs[3][:])

                yT = [pb.tile([128, TOK], BF16, name=f"yT{c}", tag=f"yT{c}")
                      for c in range(KT)]

                for c in range(KT):
                    cs = slice(c * 128, (c + 1) * 128)
                    pn = [psp.tile([128, 512], F32, name=f"pn{c}_{ih}",
                                   tag="mm") for ih in range(2)]
                    pd = [psp.tile([128, 512], F32, name=f"pd{c}_{ih}",
                                   tag="mm") for ih in range(2)]
                    for u in range(PAIRS):
                        for ih in range(2):
                            isl = slice(ih * 512, (ih + 1) * 512)
                            nc.tensor.matmul(
                                pn[ih][:], ekvg[u][:, :, cs],
                                pbr[u][:, :, isl],
                                start=(u == 0), stop=(u == PAIRS - 1),
                                perf_mode=DR)
                        for ih in range(2):
                            isl = slice(ih * 512, (ih + 1) * 512)
                            nc.tensor.matmul(
                                pd[ih][:], ekg[u][:, :, cs],
                                pbr[u][:, :, isl],
                                start=(u == 0), stop=(u == PAIRS - 1),
                                perf_mode=DR)
                    if c == 0:
                        emit_s_transpose()
                    for ih in range(2):
                        sl = slice(ih * 512, (ih + 1) * 512)
                        # num/den = (pn + 64*Snum) / (pd + 64*Sden): the
                        # einsum's 64x pre-scale and ST's 64x cancel.
                        den = pb.tile([128, 512], F32, name=f"den{c}{ih}",
                                      tag="den", bufs=3)
                        nc.scalar.activation(
                            den[:], pd[ih][:], AF.Identity,
                            bias=ST[:, c:c + 1])
                        rec = pb.tile([128, 512], F32, name=f"rec{c}{ih}",
                                      tag="rec", bufs=3)
                        nc.vector.reciprocal_approx_fast(rec[:], den[:])
                        tt = pb.tile([128, 512], F32, name=f"tt{c}{ih}",
                                     tag="tt", bufs=3)
                        nc.vector.scalar_tensor_tensor(
                            tt[:], pn[ih][:], ST[:, 8 + c:9 + c], rec[:],
                            ALU.add, ALU.mult)
                        nc.vector.tensor_mul(yT[c][:, sl], tt[:],
                                             sq_t[c][:, sl])

                # output projection: lhsT = y^T directly
                for it in range(NG):
                    po = [psp.tile([128, 512], F32, name=f"po{it}_{n}",
                                   tag="mm") for n in range(2)]
                    for c in range(KT):
                        for n in range(2):
                            nc.tensor.matmul(
                                po[n][:], yT[c][:, it * 128:(it + 1) * 128],
                                wout[c][:, n * 512:(n + 1) * 512],
                                start=(c == 0),
                                stop=(c == KT - 1 and not with_out_bias))
                    if with_out_bias:
                        for n in range(2):
                            nc.tensor.matmul(
                                po[n][:], ones1r[:],
                                bout[:, n * 512:(n + 1) * 512],
                                start=False, stop=True)
                    for n in range(2):
                        osb = pb.tile([128, 512], F32, name=f"osb{it}_{n}",
                                      tag="osb", bufs=4)
                        nc.scalar.copy(osb[:], po[n][:])
                        nc.sync.dma_start(
                            out_d[it * 128:(it + 1) * 128,
                                  n * 512:(n + 1) * 512], osb[:])

    nc.compile()
    return nc


def _prep_inputs(data, W_qkv, b_qkv, pos_bias_param, W_out, b_out):
    bf = ml_dtypes.bfloat16
    f8 = ml_dtypes.float8_e4m3
    data = np.asarray(data, np.float32)
    W_qkv = np.asarray(W_qkv, np.float32)
    b_qkv = np.asarray(b_qkv, np.float32)
    pos_bias_param = np.asarray(pos_bias_param, np.float32)
    W_out = np.asarray(W_out, np.float32)
    b_out = np.asarray(b_out, np.float32)

    with_qkv_bias = bool(np.any(b_qkv))
    with_out_bias = bool(np.any(b_out))

    wq = np.ascontiguousarray(W_qkv[:, :D]).astype(bf)
    wkv = np.ascontiguousarray(W_qkv[:, D:]).astype(bf)
    wout = W_out.astype(bf)
    # pbr[j, i] = expm1(pb[i, j]) * SCALE, fp8 (correction term of exp(pb))
    pbr_full = np.clip(np.expm1(pos_bias_param.T) * SCALE, -240.0, 240.0)
    pbr_full = pbr_full.astype(f8)

    in_maps = []
    for r in range(N_CORES):
        b, h = r // 2, r % 2
        isl = slice(h * TOK, (h + 1) * TOK)
        dT = np.ascontiguousarray(data[isl, b, :].T).astype(bf)  # [d_in, tok]
        # pair-block layout: rows u*128.. hold j-pair u; column halves are
        # the two 128-j subtiles (DoubleRow [128, 2, 1024])
        pbr_c = np.empty((TOK, 2048), f8)
        for u in range(PAIRS):
            x, hp, q = u // 4, (u // 2) % 2, u % 2
            J0 = hp * 1024 + x * 512 + q * 256
            pbr_c[u * 128:(u + 1) * 128, :TOK] = pbr_full[J0:J0 + 128, isl]
            pbr_c[u * 128:(u + 1) * 128, TOK:] = pbr_full[J0 + 128:J0 + 256, isl]
        m = {"dataT": dT, "wq": wq, "wkv": wkv, "pbr": pbr_c, "wout": wout}
        if with_qkv_bias:
            m["bkv"] = np.ascontiguousarray(b_qkv[D:]).reshape(1, 2 * D).astype(bf)
            m["bqt"] = np.ascontiguousarray(
                b_qkv[:D].reshape(KT, 128).T).astype(np.float32)
        if with_out_bias:
            m["bout"] = b_out.reshape(1, D).astype(bf)
        in_maps.append(m)
    return in_maps, with_qkv_bias, with_out_bias


def run(data, W_qkv, b_qkv, pos_bias_param, W_out, b_out, **spmd_kwargs):
    in_maps, wb, ob = _prep_inputs(data, W_qkv, b_qkv, pos_bias_param, W_out,
                                   b_out)
    key = (wb, ob)
    if key not in _cache:
        _cache[key] = build(wb, ob)
    nc = _cache[key]
    res = run_bass_kernel_spmd(nc, in_maps, core_ids=list(range(N_CORES)),
                               **spmd_kwargs)
    out = np.empty((T, B, D), np.float32)
    for r in range(N_CORES):
        b, h = r // 2, r % 2
        out[h * TOK:(h + 1) * TOK, b, :] = res.results[r]["out"]
    return out, res


def kernel(data, W_qkv, b_qkv, pos_bias_param, W_out, b_out):
    out, _ = run(data, W_qkv, b_qkv, pos_bias_param, W_out, b_out)
    return out
